# revision 25
# baseline (speedup 1.0000x reference)
"""GP posterior mean mu = K_rbf(X_test, X_train) @ alpha on 8 NeuronCores.

Block-sparse formulation.  With ell = 0.1 the RBF kernel is negligible
(K < e^-8 ~ 3e-4) for pairs further apart than ~0.4, which covers ~90% of all
(test, train) pairs on this data.  Host-side prep spatially sorts both point
sets (16 serpentine bands), partitions test into 64 chunks of 256 and train
into 128 tiles of 128, and keeps only (chunk, tile) blocks whose exact
min-pair distance is below the cutoff (~10% of blocks).  Chunks are dealt to
8 cores x 8 slots so every core runs the same padded job count -> one SPMD
program.

Math per job (train tile x test chunk), same numerics as the dense baseline:
exponent[i,j] = A[:,i] . B[:,j] via a 14-row bf16 hi/lo-split contraction
(padded to 128: sub-128 contractions throttle the PE clock to 1.2 GHz),
ScalarE exp (output-scale folded into the activation bias), then a second
matmul against bf16 hi/lo-split alpha
accumulating in PSUM.  Jobs are processed in groups of 6 so one ACT
instruction covers [128,1536] (amortizes the ~500-cycle ACT overhead).  The
8 per-slot accumulators pack into one [36,1024] PSUM tile (partition offsets
0/32 x column quarters); each slot's first matmul opens its accumulation
chain with start=True (has_written bits are per-element and stale across
executions).
"""

import numpy as np
import ml_dtypes

M = 16384
N = 16384
NCORES = 8
MC = M // NCORES          # 2048 test points per core
CT = 256                  # test points per chunk (= per slot)
NSLOT = MC // CT          # 8 slots per core
NCHUNK = M // CT          # 64 chunks total
TT = 128                  # train points per tile
NTILE = N // TT           # 128 train tiles
C = 14                    # used contraction rows of the exponent matmul
CP = 128                  # padded contraction (sub-128 contraction throttles
                          # the PE clock to 1.2 GHz -- measured, not folklore)
NBAND = 16                # serpentine sort bands
EXP_CUT = 8.0             # drop blocks where all pairs have K < e^-EXP_CUT
GRP = 6                   # jobs per ACT instruction (3 PSUM banks)

_cache = {}


def _split2(v):
    hi = v.astype(ml_dtypes.bfloat16)
    lo = (v - hi.astype(np.float64)).astype(ml_dtypes.bfloat16)
    return hi, lo


def _split3(v):
    hi = v.astype(ml_dtypes.bfloat16)
    r = v - hi.astype(np.float64)
    mid = r.astype(ml_dtypes.bfloat16)
    lo = (r - mid.astype(np.float64)).astype(ml_dtypes.bfloat16)
    return hi, mid, lo


def _band_order(X, nbands):
    """Spatial sort: nbands equal-count x-bands, serpentine by y inside."""
    n = len(X)
    ox = np.argsort(X[:, 0], kind="stable")
    per = n // nbands
    order = np.empty(n, dtype=np.int64)
    for b in range(nbands):
        seg = ox[b * per : (b + 1) * per] if b < nbands - 1 else ox[b * per :]
        oy = seg[np.argsort(X[seg, 1], kind="stable")]
        if b % 2:
            oy = oy[::-1]
        order[b * per : b * per + len(oy)] = oy
    return order


def _point_bbox_d2(P, bmin, bmax):
    dx = np.maximum(0.0, np.maximum(bmin[None, :, 0] - P[:, None, 0],
                                    P[:, None, 0] - bmax[None, :, 0]))
    dy = np.maximum(0.0, np.maximum(bmin[None, :, 1] - P[:, None, 1],
                                    P[:, None, 1] - bmax[None, :, 1]))
    return dx * dx + dy * dy


def _make_AB(xs, xt, ell2):
    """Exponent factorization: exponent = A[:, i] . B[:, j] (train i, test j)."""
    n, m = len(xt), len(xs)
    x0h, x0l = _split2(xt[:, 0])
    x1h, x1l = _split2(xt[:, 1])
    pj = -(xt[:, 0] ** 2 + xt[:, 1] ** 2) / (2.0 * ell2)
    pjh, pjm, pjl = _split3(pj)
    ones = np.ones(n, dtype=ml_dtypes.bfloat16)
    A = np.zeros((CP, n), dtype=ml_dtypes.bfloat16)
    A[:C] = np.stack(
        [ones, ones, ones, x0h, x0h, x0l, x0l, x1h, x1h, x1l, x1l, pjh, pjm, pjl]
    )

    T0 = -(xs[:, 0] ** 2 + xs[:, 1] ** 2) / (2.0 * ell2)
    T0h, T0m, T0l = _split3(T0)
    u0 = xs[:, 0] / ell2
    u0h, u0l = _split2(u0)
    u1 = xs[:, 1] / ell2
    u1h, u1l = _split2(u1)
    onesM = np.ones(m, dtype=ml_dtypes.bfloat16)
    B = np.zeros((CP, m), dtype=ml_dtypes.bfloat16)
    B[:C] = np.stack(
        [T0h, T0m, T0l, u0h, u0l, u0h, u0l, u1h, u1l, u1h, u1l, onesM, onesM, onesM]
    )
    return A, B


def _build_program(bias, njob, slot_of_job):
    import concourse.mybir as mybir
    import concourse.tile as tile
    from concourse import bacc

    fp32 = mybir.dt.float32
    bf16 = mybir.dt.bfloat16

    ngrp = njob // GRP
    first_in_slot = {s: slot_of_job.index(s) for s in set(slot_of_job)}
    last_in_slot = {s: njob - 1 - slot_of_job[::-1].index(s)
                    for s in set(slot_of_job)}

    nc = bacc.Bacc(None, target_bir_lowering=False)
    A_d = nc.declare_dram_parameter("A", [CP, njob * TT], bf16, isOutput=False)
    B_d = nc.declare_dram_parameter("B", [CP, MC], bf16, isOutput=False)
    AL_d = nc.declare_dram_parameter("AL", [TT, njob * 4], bf16, isOutput=False)
    OUT_d = nc.declare_dram_parameter("out", [36, 1024], fp32, isOutput=True)

    with tile.TileContext(nc) as tc:
        with (
            tc.tile_pool(name="singles", bufs=1) as singles,
            tc.tile_pool(name="kpool", bufs=3) as kpool,
            tc.tile_pool(name="opool", bufs=1) as opool,
            tc.tile_pool(name="pse", bufs=2, space="PSUM") as pse,
            tc.tile_pool(name="psacc", bufs=1, space="PSUM") as psacc,
        ):
            sb_B = singles.tile([CP, MC], bf16)
            sb_AL = singles.tile([TT, njob * 4], bf16)
            sb_A = singles.tile([CP, njob * TT], bf16)
            # Wave 0: job 0-5's dependencies, alone on the wire.  In-flight
            # DMA descriptors round-robin, so a tiny first piece sharing the
            # wire with the 4MB bulk lands ~4us late; instead the bulk DMAs
            # take a dependency edge on wave 0 so the first compute group's
            # data arrives ~1us after DMA issue.
            from concourse.tile import add_dep_helper
            wave0 = [
                nc.sync.dma_start(out=sb_B[:, :CT], in_=B_d[:, :CT]),
                nc.gpsimd.dma_start(out=sb_A[:, :TT], in_=A_d[:, :TT]),
                nc.gpsimd.dma_start(out=sb_A[:, TT : 3 * TT],
                                    in_=A_d[:, TT : 3 * TT]),
                nc.sync.dma_start(out=sb_A[:, 3 * TT : 6 * TT],
                                  in_=A_d[:, 3 * TT : 6 * TT]),
                nc.scalar.dma_start(out=sb_AL[:, : 48 * 4],
                                    in_=AL_d[:, : 48 * 4]),
            ]
            # A pieces flow freely (delivery barely outpaces consumption);
            # only B_rest/AL_rest (not needed until slot 1+) yield the wire
            # to wave 0 so the first compute group starts ~1us after issue.
            npiece = 6
            per = max(1, (njob - 6) // npiece)
            edges = [6 + i * per for i in range(npiece)] + [njob]
            for i in range(npiece):
                s = slice(edges[i] * TT, min(edges[i + 1], njob) * TT)
                if s.start >= s.stop:
                    continue
                eng = nc.sync if i % 2 == 0 else nc.gpsimd
                eng.dma_start(out=sb_A[:, s], in_=A_d[:, s])
            bulk = [nc.gpsimd.dma_start(out=sb_B[:, CT:], in_=B_d[:, CT:])]
            if njob > 48:
                bulk.append(nc.scalar.dma_start(out=sb_AL[:, 48 * 4 :],
                                                in_=AL_d[:, 48 * 4 :]))
            for b in bulk:
                for w in wave0:
                    add_dep_helper(b.ins, w.ins, sync=True,
                                   reason="late bulk DMA after wave0 completes")

            # 8 slot accumulators packed in one 2-bank PSUM tile:
            # slot s -> partitions 32*(s%2)..+4, cols 256*(s//2)..+256
            acc = psacc.tile([36, 1024], fp32, name="acc")

            for g in range(ngrp):
                e6 = pse.tile([128, GRP * CT], fp32)
                for q in range(GRP):
                    j = g * GRP + q
                    s = slot_of_job[j]
                    nc.tensor.matmul(
                        e6[:, q * CT : (q + 1) * CT],
                        lhsT=sb_A[:, j * TT : (j + 1) * TT],
                        rhs=sb_B[:, s * CT : (s + 1) * CT],
                        start=True,
                        stop=True,
                    )
                k6 = kpool.tile([128, GRP * CT], bf16)
                nc.scalar.activation(
                    k6, e6, mybir.ActivationFunctionType.Exp, bias=float(bias)
                )
                for q in range(GRP):
                    j = g * GRP + q
                    s = slot_of_job[j]
                    nc.tensor.matmul(
                        acc[32 * (s % 2) : 32 * (s % 2) + 4,
                            256 * (s // 2) : 256 * (s // 2) + 256],
                        lhsT=sb_AL[:, j * 4 : (j + 1) * 4],
                        rhs=k6[:, q * CT : (q + 1) * CT],
                        start=(j == first_in_slot[s]),
                        stop=(j == last_in_slot[s]),
                    )

            # two-half evacuation: the first half's deps (slots 0-3) clear
            # mid-kernel, hiding its copy + DMA under remaining compute
            for h in range(2):
                o = opool.tile([36, 512], fp32, name=f"o{h}")
                nc.vector.tensor_copy(o, acc[:, h * 512 : (h + 1) * 512])
                nc.sync.dma_start(
                    out=OUT_d[:, h * 512 : (h + 1) * 512], in_=o
                )
    nc.compile()
    return nc


def prepare(X_test, X_train, alpha, log_lengthscale, log_outputscale):
    """Host prep: sort, schedule, gather.  Returns (nc, in_maps, assemble, meta)."""
    ell = np.exp(np.float32(log_lengthscale))
    ell2 = np.float64(np.float32(ell) ** 2)
    sf = np.exp(np.float32(log_outputscale))
    sf2 = np.float64(np.float32(sf) ** 2)
    bias = np.float32(np.log(sf2))

    xs_all = X_test.astype(np.float64)
    xt_all = X_train.astype(np.float64)
    al_all = alpha.astype(np.float64)

    dcut = float(np.sqrt(2.0 * ell2 * EXP_CUT))

    ot = _band_order(xs_all, NBAND)
    orr = _band_order(xt_all, NBAND)
    xs = xs_all[ot]
    xt = xt_all[orr]
    al = al_all[orr]

    # --- block keep matrix: bbox prefilter + exact min pair distance --------
    tch = xs.reshape(NCHUNK, CT, 2)
    ttl = xt.reshape(NTILE, TT, 2)
    tmin, tmax = tch.min(1), tch.max(1)
    rmin, rmax = ttl.min(1), ttl.max(1)
    d2_tr = _point_bbox_d2(xt, tmin, tmax)
    d2_tr = d2_tr.reshape(NTILE, TT, NCHUNK).min(1)
    d2_te = _point_bbox_d2(xs, rmin, rmax)
    d2_te = d2_te.reshape(NCHUNK, CT, NTILE).min(1)
    pre = (d2_tr.T < dcut * dcut) & (d2_te < dcut * dcut)
    keep = np.zeros_like(pre)
    for ch in range(NCHUNK):
        idx = np.nonzero(pre[ch])[0]
        if len(idx) == 0:
            continue
        d2 = ((tch[ch][:, None, None, :] - ttl[idx][None, :, :, :]) ** 2).sum(-1)
        keep[ch, idx] = d2.min(axis=(0, 2)) < dcut * dcut
    cnt = keep.sum(1)
    assert cnt.min() >= 1

    # --- deal chunks to (core, slot); pad counts ---------------------------
    order = np.argsort(-cnt, kind="stable")
    T = [int(cnt[order[j * NCORES]]) for j in range(NSLOT)]
    njob = sum(T)
    pad = (-njob) % GRP
    T[-1] += pad
    njob += pad

    chunk_of = np.empty((NCORES, NSLOT), dtype=np.int64)
    for j in range(NSLOT):
        for c in range(NCORES):
            chunk_of[c, j] = order[j * NCORES + c]

    # --- exponent factor matrices & alpha tiles (sorted order) -------------
    A, B = _make_AB(xs, xt, ell2)
    arh, arl = _split2(al[:, 0])
    aih, ail = _split2(al[:, 1])
    AL = np.stack([arh, arl, aih, ail], axis=1)      # (N, 4) bf16
    AL = np.ascontiguousarray(AL).reshape(NTILE, TT, 4)

    # --- per-core gathers --------------------------------------------------
    in_maps = []
    slot_of_job = []
    for j in range(NSLOT):
        slot_of_job += [j] * T[j]
    for c in range(NCORES):
        A_g = np.zeros((CP, njob * TT), dtype=ml_dtypes.bfloat16)
        AL_g = np.zeros((TT, njob * 4), dtype=ml_dtypes.bfloat16)
        B_c = np.empty((CP, MC), dtype=ml_dtypes.bfloat16)
        ji = 0
        for j in range(NSLOT):
            ch = chunk_of[c, j]
            B_c[:, j * CT : (j + 1) * CT] = B[:, ch * CT : (ch + 1) * CT]
            tiles = np.nonzero(keep[ch])[0]
            for t in tiles:
                A_g[:, ji * TT : (ji + 1) * TT] = A[:, t * TT : (t + 1) * TT]
                AL_g[:, ji * 4 : (ji + 1) * 4] = AL[t]
                ji += 1
            ji += T[j] - len(tiles)  # dummy jobs stay zero
        assert ji == njob
        in_maps.append({"A": A_g, "B": B_c, "AL": AL_g})

    key = ("v4", float(bias), njob, tuple(T))
    if key not in _cache:
        _cache[key] = _build_program(bias, njob, slot_of_job)
    nc = _cache[key]

    def assemble(results):
        out = np.empty((M, 2), dtype=np.float32)
        for c in range(NCORES):
            o = results[c]["out"]  # (36, 1024)
            for j in range(NSLOT):
                g, q = j % 2, j // 2
                blk = o[32 * g : 32 * g + 4, 256 * q : 256 * q + 256]
                ch = chunk_of[c, j]
                rows = ot[ch * CT : (ch + 1) * CT]
                out[rows, 0] = blk[0] + blk[1]
                out[rows, 1] = blk[2] + blk[3]
        return out

    return nc, in_maps, assemble, slot_of_job


def simulate(nc_unused, in_maps, slot_of_job, bias):
    """Numpy emulation of the device program (for schedule/gather checks)."""
    results = []
    njob = len(slot_of_job)
    for c in range(NCORES):
        A_g = in_maps[c]["A"].astype(np.float32)
        B_c = in_maps[c]["B"].astype(np.float32)
        AL_g = in_maps[c]["AL"].astype(np.float32)
        o = np.zeros((36, 1024), dtype=np.float32)
        for j in range(njob):
            s = slot_of_job[j]
            e = A_g[:, j * TT : (j + 1) * TT].T @ B_c[:, s * CT : (s + 1) * CT]
            k = np.exp(e + bias).astype(ml_dtypes.bfloat16).astype(np.float32)
            contrib = AL_g[:, j * 4 : (j + 1) * 4].T @ k  # (4, CT)
            o[32 * (s % 2) : 32 * (s % 2) + 4,
              256 * (s // 2) : 256 * (s // 2) + 256] += contrib
        results.append({"out": o})
    return results


def kernel(X_test, X_train, alpha, log_lengthscale, log_outputscale):
    from concourse.bass_utils import run_bass_kernel_spmd

    nc, in_maps, assemble, _ = prepare(
        X_test, X_train, alpha, log_lengthscale, log_outputscale
    )
    res = run_bass_kernel_spmd(nc, in_maps, list(range(NCORES)))
    return assemble(res.results)


# revision 32
# speedup vs baseline: 1.0527x; 1.0527x over previous
"""GP posterior mean mu = K_rbf(X_test, X_train) @ alpha on 8 NeuronCores.

Block-sparse formulation.  With ell = 0.1 the RBF kernel is negligible
(K < e^-8 ~ 3e-4) for pairs further apart than ~0.4, which covers ~90% of all
(test, train) pairs on this data.  Host-side prep spatially sorts both point
sets (16 serpentine bands), partitions test into 64 chunks of 256 and train
into 128 tiles of 128, and keeps only (chunk, tile) blocks whose exact
min-pair distance is below the cutoff (~10% of blocks).  Chunks are dealt to
8 cores x 8 slots so every core runs the same padded job count -> one SPMD
program.

Math per job (train tile x test chunk), same numerics as the dense baseline:
exponent[i,j] = A[:,i] . B[:,j] via a 14-row bf16 hi/lo-split contraction
(padded to 128: sub-128 contractions throttle the PE clock to 1.2 GHz),
ScalarE exp (output-scale folded into the activation bias), then a second
matmul against bf16 hi/lo-split alpha
accumulating in PSUM.  Jobs are processed in groups of 6 so one ACT
instruction covers [128,1536] (amortizes the ~500-cycle ACT overhead).  The
8 per-slot accumulators pack into one [36,1024] PSUM tile (partition offsets
0/32 x column quarters); each slot's first matmul opens its accumulation
chain with start=True (has_written bits are per-element and stale across
executions).
"""

import numpy as np
import ml_dtypes

M = 16384
N = 16384
NCORES = 8
MC = M // NCORES          # 2048 test points per core
CT = 256                  # test points per chunk (= per slot)
NSLOT = MC // CT          # 8 slots per core
NCHUNK = M // CT          # 64 chunks total
TT = 128                  # train points per tile
NTILE = N // TT           # 128 train tiles
C = 14                    # used contraction rows of the exponent matmul
CP = 128                  # padded contraction (sub-128 contraction throttles
                          # the PE clock to 1.2 GHz -- measured, not folklore)
NBAND = 16                # serpentine sort bands
EXP_CUT = 8.0             # drop blocks where all pairs have K < e^-EXP_CUT
GRP = 6                   # jobs per ACT instruction (3 PSUM banks)

_cache = {}


def _split2(v):
    hi = v.astype(ml_dtypes.bfloat16)
    lo = (v - hi.astype(np.float64)).astype(ml_dtypes.bfloat16)
    return hi, lo


def _split3(v):
    hi = v.astype(ml_dtypes.bfloat16)
    r = v - hi.astype(np.float64)
    mid = r.astype(ml_dtypes.bfloat16)
    lo = (r - mid.astype(np.float64)).astype(ml_dtypes.bfloat16)
    return hi, mid, lo


def _band_order(X, nbands):
    """Spatial sort: nbands equal-count x-bands, serpentine by y inside."""
    n = len(X)
    ox = np.argsort(X[:, 0], kind="stable")
    per = n // nbands
    order = np.empty(n, dtype=np.int64)
    for b in range(nbands):
        seg = ox[b * per : (b + 1) * per] if b < nbands - 1 else ox[b * per :]
        oy = seg[np.argsort(X[seg, 1], kind="stable")]
        if b % 2:
            oy = oy[::-1]
        order[b * per : b * per + len(oy)] = oy
    return order


def _point_bbox_d2(P, bmin, bmax):
    dx = np.maximum(0.0, np.maximum(bmin[None, :, 0] - P[:, None, 0],
                                    P[:, None, 0] - bmax[None, :, 0]))
    dy = np.maximum(0.0, np.maximum(bmin[None, :, 1] - P[:, None, 1],
                                    P[:, None, 1] - bmax[None, :, 1]))
    return dx * dx + dy * dy


def _make_AB(xs, xt, ell2):
    """Exponent factorization: exponent = A[:, i] . B[:, j] (train i, test j).

    Only the C=14 real contraction rows are materialized; the device zero-
    fills rows C..CP with a DVE memset (shipping the zero padding would be
    7x the DMA bytes).
    """
    n, m = len(xt), len(xs)
    x0h, x0l = _split2(xt[:, 0])
    x1h, x1l = _split2(xt[:, 1])
    pj = -(xt[:, 0] ** 2 + xt[:, 1] ** 2) / (2.0 * ell2)
    pjh, pjm, pjl = _split3(pj)
    ones = np.ones(n, dtype=ml_dtypes.bfloat16)
    A = np.stack(
        [ones, ones, ones, x0h, x0h, x0l, x0l, x1h, x1h, x1l, x1l, pjh, pjm, pjl]
    ).astype(ml_dtypes.bfloat16)

    T0 = -(xs[:, 0] ** 2 + xs[:, 1] ** 2) / (2.0 * ell2)
    T0h, T0m, T0l = _split3(T0)
    u0 = xs[:, 0] / ell2
    u0h, u0l = _split2(u0)
    u1 = xs[:, 1] / ell2
    u1h, u1l = _split2(u1)
    onesM = np.ones(m, dtype=ml_dtypes.bfloat16)
    B = np.stack(
        [T0h, T0m, T0l, u0h, u0l, u0h, u0l, u1h, u1l, u1h, u1l, onesM, onesM, onesM]
    ).astype(ml_dtypes.bfloat16)
    return A, B


def _build_program(bias, njob, slot_of_job):
    import concourse.mybir as mybir
    import concourse.tile as tile
    from concourse import bacc

    fp32 = mybir.dt.float32
    bf16 = mybir.dt.bfloat16

    ngrp = njob // GRP
    first_in_slot = {s: slot_of_job.index(s) for s in set(slot_of_job)}
    last_in_slot = {s: njob - 1 - slot_of_job[::-1].index(s)
                    for s in set(slot_of_job)}

    nc = bacc.Bacc(None, target_bir_lowering=False)
    A_d = nc.declare_dram_parameter("A", [C, njob * TT], bf16, isOutput=False)
    B_d = nc.declare_dram_parameter("B", [C, MC], bf16, isOutput=False)
    AL_d = nc.declare_dram_parameter("AL", [TT, njob * 4], bf16, isOutput=False)
    OUT_d = nc.declare_dram_parameter("out", [36, 1024], fp32, isOutput=True)

    with tile.TileContext(nc) as tc:
        with (
            tc.tile_pool(name="singles", bufs=1) as singles,
            tc.tile_pool(name="kpool", bufs=3) as kpool,
            tc.tile_pool(name="opool", bufs=1) as opool,
            tc.tile_pool(name="pse", bufs=2, space="PSUM") as pse,
            tc.tile_pool(name="psacc", bufs=1, space="PSUM") as psacc,
        ):
            sb_B = singles.tile([CP, MC], bf16)
            sb_AL = singles.tile([TT, njob * 4], bf16)
            sb_A = singles.tile([CP, njob * TT], bf16)
            # Only rows 0:C come over the wire (~0.6MB total).  Rows C:CP
            # must be zero, but engine ops need 32-aligned partition bases,
            # so memset the full 128-partition region first and let the DMA
            # overwrite rows 0:C (piece-aligned to keep dependencies
            # fine-grained).
            nc.vector.memset(sb_B, 0)
            nc.sync.dma_start(out=sb_B[:C, :], in_=B_d[:])
            nc.scalar.dma_start(out=sb_AL[:, : 48 * 4], in_=AL_d[:, : 48 * 4])
            if njob > 48:
                nc.scalar.dma_start(out=sb_AL[:, 48 * 4 :], in_=AL_d[:, 48 * 4 :])
            npiece = 5
            per = max(1, (njob - 6) // npiece)
            edges = [0, 6] + [6 + (i + 1) * per for i in range(npiece - 1)] + [njob]
            for i in range(len(edges) - 1):
                s = slice(edges[i] * TT, min(edges[i + 1], njob) * TT)
                if s.start >= s.stop:
                    continue
                nc.vector.memset(sb_A[:, s], 0)
                eng = nc.gpsimd if i % 2 == 0 else nc.sync
                eng.dma_start(out=sb_A[:C, s], in_=A_d[:, s])

            # 8 slot accumulators packed in one 2-bank PSUM tile:
            # slot s -> partitions 32*(s%2)..+4, cols 256*(s//2)..+256
            acc = psacc.tile([36, 1024], fp32, name="acc")

            for g in range(ngrp):
                e6 = pse.tile([128, GRP * CT], fp32)
                for q in range(GRP):
                    j = g * GRP + q
                    s = slot_of_job[j]
                    nc.tensor.matmul(
                        e6[:, q * CT : (q + 1) * CT],
                        lhsT=sb_A[:, j * TT : (j + 1) * TT],
                        rhs=sb_B[:, s * CT : (s + 1) * CT],
                        start=True,
                        stop=True,
                    )
                k6 = kpool.tile([128, GRP * CT], bf16)
                nc.scalar.activation(
                    k6, e6, mybir.ActivationFunctionType.Exp, bias=float(bias)
                )
                for q in range(GRP):
                    j = g * GRP + q
                    s = slot_of_job[j]
                    nc.tensor.matmul(
                        acc[32 * (s % 2) : 32 * (s % 2) + 4,
                            256 * (s // 2) : 256 * (s // 2) + 256],
                        lhsT=sb_AL[:, j * 4 : (j + 1) * 4],
                        rhs=k6[:, q * CT : (q + 1) * CT],
                        start=(j == first_in_slot[s]),
                        stop=(j == last_in_slot[s]),
                    )

            # two-half evacuation: the first half's deps (slots 0-3) clear
            # mid-kernel, hiding its copy + DMA under remaining compute
            for h in range(2):
                o = opool.tile([36, 512], fp32, name=f"o{h}")
                nc.vector.tensor_copy(o, acc[:, h * 512 : (h + 1) * 512])
                nc.sync.dma_start(
                    out=OUT_d[:, h * 512 : (h + 1) * 512], in_=o
                )
    nc.compile()
    return nc


def prepare(X_test, X_train, alpha, log_lengthscale, log_outputscale):
    """Host prep: sort, schedule, gather.  Returns (nc, in_maps, assemble, meta)."""
    ell = np.exp(np.float32(log_lengthscale))
    ell2 = np.float64(np.float32(ell) ** 2)
    sf = np.exp(np.float32(log_outputscale))
    sf2 = np.float64(np.float32(sf) ** 2)
    bias = np.float32(np.log(sf2))

    xs_all = X_test.astype(np.float64)
    xt_all = X_train.astype(np.float64)
    al_all = alpha.astype(np.float64)

    dcut = float(np.sqrt(2.0 * ell2 * EXP_CUT))

    ot = _band_order(xs_all, NBAND)
    orr = _band_order(xt_all, NBAND)
    xs = xs_all[ot]
    xt = xt_all[orr]
    al = al_all[orr]

    # --- block keep matrix: bbox prefilter + exact min pair distance --------
    tch = xs.reshape(NCHUNK, CT, 2)
    ttl = xt.reshape(NTILE, TT, 2)
    tmin, tmax = tch.min(1), tch.max(1)
    rmin, rmax = ttl.min(1), ttl.max(1)
    d2_tr = _point_bbox_d2(xt, tmin, tmax)
    d2_tr = d2_tr.reshape(NTILE, TT, NCHUNK).min(1)
    d2_te = _point_bbox_d2(xs, rmin, rmax)
    d2_te = d2_te.reshape(NCHUNK, CT, NTILE).min(1)
    pre = (d2_tr.T < dcut * dcut) & (d2_te < dcut * dcut)
    keep = np.zeros_like(pre)
    for ch in range(NCHUNK):
        idx = np.nonzero(pre[ch])[0]
        if len(idx) == 0:
            continue
        d2 = ((tch[ch][:, None, None, :] - ttl[idx][None, :, :, :]) ** 2).sum(-1)
        keep[ch, idx] = d2.min(axis=(0, 2)) < dcut * dcut
    cnt = keep.sum(1)

    # --- deal chunks to (core, slot); pad counts ---------------------------
    order = np.argsort(-cnt, kind="stable")
    T = [max(int(cnt[order[j * NCORES]]), 1) for j in range(NSLOT)]
    njob = sum(T)
    pad = (-njob) % GRP
    T[-1] += pad
    njob += pad

    chunk_of = np.empty((NCORES, NSLOT), dtype=np.int64)
    for j in range(NSLOT):
        for c in range(NCORES):
            chunk_of[c, j] = order[j * NCORES + c]

    # --- exponent factor matrices & alpha tiles (sorted order) -------------
    A, B = _make_AB(xs, xt, ell2)
    arh, arl = _split2(al[:, 0])
    aih, ail = _split2(al[:, 1])
    AL = np.stack([arh, arl, aih, ail], axis=1)      # (N, 4) bf16
    AL = np.ascontiguousarray(AL).reshape(NTILE, TT, 4)

    # --- per-core gathers --------------------------------------------------
    in_maps = []
    slot_of_job = []
    for j in range(NSLOT):
        slot_of_job += [j] * T[j]
    for c in range(NCORES):
        A_g = np.zeros((C, njob * TT), dtype=ml_dtypes.bfloat16)
        AL_g = np.zeros((TT, njob * 4), dtype=ml_dtypes.bfloat16)
        B_c = np.empty((C, MC), dtype=ml_dtypes.bfloat16)
        ji = 0
        for j in range(NSLOT):
            ch = chunk_of[c, j]
            B_c[:, j * CT : (j + 1) * CT] = B[:, ch * CT : (ch + 1) * CT]
            tiles = np.nonzero(keep[ch])[0]
            for t in tiles:
                A_g[:, ji * TT : (ji + 1) * TT] = A[:, t * TT : (t + 1) * TT]
                AL_g[:, ji * 4 : (ji + 1) * 4] = AL[t]
                ji += 1
            ji += T[j] - len(tiles)  # dummy jobs stay zero
        assert ji == njob
        in_maps.append({"A": A_g, "B": B_c, "AL": AL_g})

    key = ("v9", float(bias), njob, tuple(T))
    if key not in _cache:
        _cache[key] = _build_program(bias, njob, slot_of_job)
    nc = _cache[key]

    def assemble(results):
        out = np.empty((M, 2), dtype=np.float32)
        for c in range(NCORES):
            o = results[c]["out"]  # (36, 1024)
            for j in range(NSLOT):
                g, q = j % 2, j // 2
                blk = o[32 * g : 32 * g + 4, 256 * q : 256 * q + 256]
                ch = chunk_of[c, j]
                rows = ot[ch * CT : (ch + 1) * CT]
                out[rows, 0] = blk[0] + blk[1]
                out[rows, 1] = blk[2] + blk[3]
        return out

    return nc, in_maps, assemble, slot_of_job


def simulate(nc_unused, in_maps, slot_of_job, bias):
    """Numpy emulation of the device program (for schedule/gather checks)."""
    results = []
    njob = len(slot_of_job)
    for c in range(NCORES):
        A_g = in_maps[c]["A"].astype(np.float32)
        B_c = in_maps[c]["B"].astype(np.float32)
        AL_g = in_maps[c]["AL"].astype(np.float32)
        o = np.zeros((36, 1024), dtype=np.float32)
        for j in range(njob):
            s = slot_of_job[j]
            e = A_g[:, j * TT : (j + 1) * TT].T @ B_c[:, s * CT : (s + 1) * CT]
            k = np.exp(e + bias).astype(ml_dtypes.bfloat16).astype(np.float32)
            contrib = AL_g[:, j * 4 : (j + 1) * 4].T @ k  # (4, CT)
            o[32 * (s % 2) : 32 * (s % 2) + 4,
              256 * (s // 2) : 256 * (s // 2) + 256] += contrib
        results.append({"out": o})
    return results


def kernel(X_test, X_train, alpha, log_lengthscale, log_outputscale):
    from concourse.bass_utils import run_bass_kernel_spmd

    nc, in_maps, assemble, _ = prepare(
        X_test, X_train, alpha, log_lengthscale, log_outputscale
    )
    res = run_bass_kernel_spmd(nc, in_maps, list(range(NCORES)))
    return assemble(res.results)


# revision 36
# speedup vs baseline: 1.0965x; 1.0416x over previous
"""GP posterior mean mu = K_rbf(X_test, X_train) @ alpha on 8 NeuronCores.

Block-sparse formulation.  With ell = 0.1 the RBF kernel is negligible
(K < e^-8 ~ 3e-4) for pairs further apart than ~0.4, which covers ~90% of all
(test, train) pairs on this data.  Host-side prep spatially sorts both point
sets (16 serpentine bands), partitions test into 64 chunks of 256 and train
into 128 tiles of 128, and keeps only (chunk, tile) blocks whose exact
min-pair distance is below the cutoff (~10% of blocks).  Chunks are dealt to
8 cores x 8 slots so every core runs the same padded job count -> one SPMD
program.

Math per job (train tile x test chunk), same numerics as the dense baseline:
exponent[i,j] = A[:,i] . B[:,j] via a 14-row bf16 hi/lo-split contraction
(padded to 128: sub-128 contractions throttle the PE clock to 1.2 GHz),
ScalarE exp (output-scale folded into the activation bias), then a second
matmul against bf16 hi/lo-split alpha
accumulating in PSUM.  Jobs are processed in groups of 6 so one ACT
instruction covers [128,1536] (amortizes the ~500-cycle ACT overhead).  The
8 per-slot accumulators pack into one [36,1024] PSUM tile (partition offsets
0/32 x column quarters); each slot's first matmul opens its accumulation
chain with start=True (has_written bits are per-element and stale across
executions).
"""

import numpy as np
import ml_dtypes

M = 16384
N = 16384
NCORES = 8
MC = M // NCORES          # 2048 test points per core
CT = 256                  # test points per chunk (= per slot)
NSLOT = MC // CT          # 8 slots per core
NCHUNK = M // CT          # 64 chunks total
TT = 128                  # train points per tile
NTILE = N // TT           # 128 train tiles
C = 14                    # used contraction rows of the exponent matmul
CP = 128                  # padded contraction (sub-128 contraction throttles
                          # the PE clock to 1.2 GHz -- measured, not folklore)
NBAND = 16                # serpentine sort bands
EXP_CUT = 8.0             # drop blocks where all pairs have K < e^-EXP_CUT
GRP = 6                   # jobs per ACT instruction (3 PSUM banks)

_cache = {}


def _split2(v):
    hi = v.astype(ml_dtypes.bfloat16)
    lo = (v - hi.astype(np.float64)).astype(ml_dtypes.bfloat16)
    return hi, lo


def _split3(v):
    hi = v.astype(ml_dtypes.bfloat16)
    r = v - hi.astype(np.float64)
    mid = r.astype(ml_dtypes.bfloat16)
    lo = (r - mid.astype(np.float64)).astype(ml_dtypes.bfloat16)
    return hi, mid, lo


def _band_order(X, nbands):
    """Spatial sort: nbands equal-count x-bands, serpentine by y inside."""
    n = len(X)
    ox = np.argsort(X[:, 0], kind="stable")
    per = n // nbands
    order = np.empty(n, dtype=np.int64)
    for b in range(nbands):
        seg = ox[b * per : (b + 1) * per] if b < nbands - 1 else ox[b * per :]
        oy = seg[np.argsort(X[seg, 1], kind="stable")]
        if b % 2:
            oy = oy[::-1]
        order[b * per : b * per + len(oy)] = oy
    return order


def _point_bbox_d2(P, bmin, bmax):
    dx = np.maximum(0.0, np.maximum(bmin[None, :, 0] - P[:, None, 0],
                                    P[:, None, 0] - bmax[None, :, 0]))
    dy = np.maximum(0.0, np.maximum(bmin[None, :, 1] - P[:, None, 1],
                                    P[:, None, 1] - bmax[None, :, 1]))
    return dx * dx + dy * dy


def _make_AB(xs, xt, ell2):
    """Exponent factorization: exponent = A[:, i] . B[:, j] (train i, test j).

    Only the C=14 real contraction rows are materialized; the device zero-
    fills rows C..CP with a DVE memset (shipping the zero padding would be
    7x the DMA bytes).
    """
    n, m = len(xt), len(xs)
    x0h, x0l = _split2(xt[:, 0])
    x1h, x1l = _split2(xt[:, 1])
    pj = -(xt[:, 0] ** 2 + xt[:, 1] ** 2) / (2.0 * ell2)
    pjh, pjm, pjl = _split3(pj)
    ones = np.ones(n, dtype=ml_dtypes.bfloat16)
    A = np.stack(
        [ones, ones, ones, x0h, x0h, x0l, x0l, x1h, x1h, x1l, x1l, pjh, pjm, pjl]
    ).astype(ml_dtypes.bfloat16)

    T0 = -(xs[:, 0] ** 2 + xs[:, 1] ** 2) / (2.0 * ell2)
    T0h, T0m, T0l = _split3(T0)
    u0 = xs[:, 0] / ell2
    u0h, u0l = _split2(u0)
    u1 = xs[:, 1] / ell2
    u1h, u1l = _split2(u1)
    onesM = np.ones(m, dtype=ml_dtypes.bfloat16)
    B = np.stack(
        [T0h, T0m, T0l, u0h, u0l, u0h, u0l, u1h, u1l, u1h, u1l, onesM, onesM, onesM]
    ).astype(ml_dtypes.bfloat16)
    return A, B


def _build_program(bias, njob, slot_of_job):
    import concourse.mybir as mybir
    import concourse.tile as tile
    from concourse import bacc

    fp32 = mybir.dt.float32
    bf16 = mybir.dt.bfloat16

    ngrp = njob // GRP
    first_in_slot = {s: slot_of_job.index(s) for s in set(slot_of_job)}
    last_in_slot = {s: njob - 1 - slot_of_job[::-1].index(s)
                    for s in set(slot_of_job)}

    nc = bacc.Bacc(None, target_bir_lowering=False)
    A_d = nc.declare_dram_parameter("A", [C, njob * TT], bf16, isOutput=False)
    B_d = nc.declare_dram_parameter("B", [CP, MC], bf16, isOutput=False)
    AL_d = nc.declare_dram_parameter("AL", [TT, njob * 4], bf16, isOutput=False)
    OUT_d = nc.declare_dram_parameter("out", [36, 1024], fp32, isOutput=True)

    with tile.TileContext(nc) as tc:
        with (
            tc.tile_pool(name="singles", bufs=1) as singles,
            tc.tile_pool(name="apool", bufs=1) as apool,
            tc.tile_pool(name="kpool", bufs=3) as kpool,
            tc.tile_pool(name="opool", bufs=1) as opool,
            tc.tile_pool(name="pse", bufs=2, space="PSUM") as pse,
            tc.tile_pool(name="psacc", bufs=1, space="PSUM") as psacc,
        ):
            sb_B = singles.tile([CP, MC], bf16)
            sb_AL = singles.tile([TT, njob * 4], bf16)
            # Rotating pre-zeroed staging for the A tiles: rows C:CP are
            # zeroed once per slot; each group's DMA then writes only the C
            # real rows (shipping the zero padding would be 9x the bytes, and
            # per-piece memsets at DVE 1x rate could not keep up).
            NSLOTS_A = 4
            a_slots = [apool.tile([CP, GRP * TT], bf16, name=f"a{i}")
                       for i in range(NSLOTS_A)]
            for i, a in enumerate(a_slots):
                (nc.vector if i % 2 == 0 else nc.gpsimd).memset(a, 0)
            nc.sync.dma_start(out=sb_B[:, :CT], in_=B_d[:, :CT])
            nc.gpsimd.dma_start(out=a_slots[0][:C, :],
                                in_=A_d[:, : GRP * TT])
            nc.scalar.dma_start(out=sb_AL[:, : 48 * 4], in_=AL_d[:, : 48 * 4])
            nc.sync.dma_start(out=sb_B[:, CT:], in_=B_d[:, CT:])
            if njob > 48:
                nc.scalar.dma_start(out=sb_AL[:, 48 * 4 :], in_=AL_d[:, 48 * 4 :])

            # 8 slot accumulators packed in one 2-bank PSUM tile:
            # slot s -> partitions 32*(s%2)..+4, cols 256*(s//2)..+256
            acc = psacc.tile([36, 1024], fp32, name="acc")

            for g in range(ngrp):
                a6 = a_slots[g % NSLOTS_A]
                if g > 0:
                    eng = nc.gpsimd if g % 2 == 0 else nc.sync
                    eng.dma_start(
                        out=a6[:C, :],
                        in_=A_d[:, g * GRP * TT : (g + 1) * GRP * TT],
                    )
                e6 = pse.tile([128, GRP * CT], fp32)
                for q in range(GRP):
                    j = g * GRP + q
                    s = slot_of_job[j]
                    nc.tensor.matmul(
                        e6[:, q * CT : (q + 1) * CT],
                        lhsT=a6[:, q * TT : (q + 1) * TT],
                        rhs=sb_B[:, s * CT : (s + 1) * CT],
                        start=True,
                        stop=True,
                    )
                k6 = kpool.tile([128, GRP * CT], bf16)
                nc.scalar.activation(
                    k6, e6, mybir.ActivationFunctionType.Exp, bias=float(bias)
                )
                for q in range(GRP):
                    j = g * GRP + q
                    s = slot_of_job[j]
                    nc.tensor.matmul(
                        acc[32 * (s % 2) : 32 * (s % 2) + 4,
                            256 * (s // 2) : 256 * (s // 2) + 256],
                        lhsT=sb_AL[:, j * 4 : (j + 1) * 4],
                        rhs=k6[:, q * CT : (q + 1) * CT],
                        start=(j == first_in_slot[s]),
                        stop=(j == last_in_slot[s]),
                    )

            # two-half evacuation: the first half's deps (slots 0-3) clear
            # mid-kernel, hiding its copy + DMA under remaining compute
            for h in range(2):
                o = opool.tile([36, 512], fp32, name=f"o{h}")
                nc.vector.tensor_copy(o, acc[:, h * 512 : (h + 1) * 512])
                nc.sync.dma_start(
                    out=OUT_d[:, h * 512 : (h + 1) * 512], in_=o
                )
    nc.compile()
    return nc


def prepare(X_test, X_train, alpha, log_lengthscale, log_outputscale):
    """Host prep: sort, schedule, gather.  Returns (nc, in_maps, assemble, meta)."""
    ell = np.exp(np.float32(log_lengthscale))
    ell2 = np.float64(np.float32(ell) ** 2)
    sf = np.exp(np.float32(log_outputscale))
    sf2 = np.float64(np.float32(sf) ** 2)
    bias = np.float32(np.log(sf2))

    xs_all = X_test.astype(np.float64)
    xt_all = X_train.astype(np.float64)
    al_all = alpha.astype(np.float64)

    dcut = float(np.sqrt(2.0 * ell2 * EXP_CUT))

    ot = _band_order(xs_all, NBAND)
    orr = _band_order(xt_all, NBAND)
    xs = xs_all[ot]
    xt = xt_all[orr]
    al = al_all[orr]

    # --- block keep matrix: bbox prefilter + exact min pair distance --------
    tch = xs.reshape(NCHUNK, CT, 2)
    ttl = xt.reshape(NTILE, TT, 2)
    tmin, tmax = tch.min(1), tch.max(1)
    rmin, rmax = ttl.min(1), ttl.max(1)
    d2_tr = _point_bbox_d2(xt, tmin, tmax)
    d2_tr = d2_tr.reshape(NTILE, TT, NCHUNK).min(1)
    d2_te = _point_bbox_d2(xs, rmin, rmax)
    d2_te = d2_te.reshape(NCHUNK, CT, NTILE).min(1)
    pre = (d2_tr.T < dcut * dcut) & (d2_te < dcut * dcut)
    keep = np.zeros_like(pre)
    for ch in range(NCHUNK):
        idx = np.nonzero(pre[ch])[0]
        if len(idx) == 0:
            continue
        d2 = ((tch[ch][:, None, None, :] - ttl[idx][None, :, :, :]) ** 2).sum(-1)
        keep[ch, idx] = d2.min(axis=(0, 2)) < dcut * dcut
    cnt = keep.sum(1)

    # --- deal chunks to (core, slot); pad counts ---------------------------
    order = np.argsort(-cnt, kind="stable")
    T = [max(int(cnt[order[j * NCORES]]), 1) for j in range(NSLOT)]
    njob = sum(T)
    pad = (-njob) % GRP
    T[-1] += pad
    njob += pad

    chunk_of = np.empty((NCORES, NSLOT), dtype=np.int64)
    for j in range(NSLOT):
        for c in range(NCORES):
            chunk_of[c, j] = order[j * NCORES + c]

    # --- exponent factor matrices & alpha tiles (sorted order) -------------
    A, B = _make_AB(xs, xt, ell2)
    arh, arl = _split2(al[:, 0])
    aih, ail = _split2(al[:, 1])
    AL = np.stack([arh, arl, aih, ail], axis=1)      # (N, 4) bf16
    AL = np.ascontiguousarray(AL).reshape(NTILE, TT, 4)

    # --- per-core gathers --------------------------------------------------
    in_maps = []
    slot_of_job = []
    for j in range(NSLOT):
        slot_of_job += [j] * T[j]
    for c in range(NCORES):
        A_g = np.zeros((C, njob * TT), dtype=ml_dtypes.bfloat16)
        AL_g = np.zeros((TT, njob * 4), dtype=ml_dtypes.bfloat16)
        B_c = np.zeros((CP, MC), dtype=ml_dtypes.bfloat16)
        ji = 0
        for j in range(NSLOT):
            ch = chunk_of[c, j]
            B_c[:C, j * CT : (j + 1) * CT] = B[:, ch * CT : (ch + 1) * CT]
            tiles = np.nonzero(keep[ch])[0]
            for t in tiles:
                A_g[:, ji * TT : (ji + 1) * TT] = A[:, t * TT : (t + 1) * TT]
                AL_g[:, ji * 4 : (ji + 1) * 4] = AL[t]
                ji += 1
            ji += T[j] - len(tiles)  # dummy jobs stay zero
        assert ji == njob
        in_maps.append({"A": A_g, "B": B_c, "AL": AL_g})

    key = ("v9", float(bias), njob, tuple(T))
    if key not in _cache:
        _cache[key] = _build_program(bias, njob, slot_of_job)
    nc = _cache[key]

    def assemble(results):
        out = np.empty((M, 2), dtype=np.float32)
        for c in range(NCORES):
            o = results[c]["out"]  # (36, 1024)
            for j in range(NSLOT):
                g, q = j % 2, j // 2
                blk = o[32 * g : 32 * g + 4, 256 * q : 256 * q + 256]
                ch = chunk_of[c, j]
                rows = ot[ch * CT : (ch + 1) * CT]
                out[rows, 0] = blk[0] + blk[1]
                out[rows, 1] = blk[2] + blk[3]
        return out

    return nc, in_maps, assemble, slot_of_job


def simulate(nc_unused, in_maps, slot_of_job, bias):
    """Numpy emulation of the device program (for schedule/gather checks)."""
    results = []
    njob = len(slot_of_job)
    for c in range(NCORES):
        A_g = in_maps[c]["A"].astype(np.float32)
        B_c = in_maps[c]["B"].astype(np.float32)
        AL_g = in_maps[c]["AL"].astype(np.float32)
        o = np.zeros((36, 1024), dtype=np.float32)
        for j in range(njob):
            s = slot_of_job[j]
            e = (A_g[:, j * TT : (j + 1) * TT].T
                 @ B_c[: A_g.shape[0], s * CT : (s + 1) * CT])
            k = np.exp(e + bias).astype(ml_dtypes.bfloat16).astype(np.float32)
            contrib = AL_g[:, j * 4 : (j + 1) * 4].T @ k  # (4, CT)
            o[32 * (s % 2) : 32 * (s % 2) + 4,
              256 * (s // 2) : 256 * (s // 2) + 256] += contrib
        results.append({"out": o})
    return results


def kernel(X_test, X_train, alpha, log_lengthscale, log_outputscale):
    from concourse.bass_utils import run_bass_kernel_spmd

    nc, in_maps, assemble, _ = prepare(
        X_test, X_train, alpha, log_lengthscale, log_outputscale
    )
    res = run_bass_kernel_spmd(nc, in_maps, list(range(NCORES)))
    return assemble(res.results)


# revision 39
# speedup vs baseline: 1.1378x; 1.0377x over previous
"""GP posterior mean mu = K_rbf(X_test, X_train) @ alpha on 8 NeuronCores.

Block-sparse formulation.  With ell = 0.1 the RBF kernel is negligible
(K < e^-8 ~ 3e-4) for pairs further apart than ~0.4, which covers ~90% of all
(test, train) pairs on this data.  Host-side prep spatially sorts both point
sets (16 serpentine bands), partitions test into 64 chunks of 256 and train
into 128 tiles of 128, and keeps only (chunk, tile) blocks whose exact
min-pair distance is below the cutoff (~10% of blocks).  Chunks are dealt to
8 cores x 8 slots so every core runs the same padded job count -> one SPMD
program.

Math per job (train tile x test chunk), same numerics as the dense baseline:
exponent[i,j] = A[:,i] . B[:,j] via a 14-row bf16 hi/lo-split contraction
(padded to 128: sub-128 contractions throttle the PE clock to 1.2 GHz),
ScalarE exp (output-scale folded into the activation bias), then a second
matmul against bf16 hi/lo-split alpha
accumulating in PSUM.  Jobs are processed in groups of 6 so one ACT
instruction covers [128,1536] (amortizes the ~500-cycle ACT overhead).  The
8 per-slot accumulators pack into one [36,1024] PSUM tile (partition offsets
0/32 x column quarters); each slot's first matmul opens its accumulation
chain with start=True (has_written bits are per-element and stale across
executions).
"""

import numpy as np
import ml_dtypes

M = 16384
N = 16384
NCORES = 8
MC = M // NCORES          # 2048 test points per core
CT = 256                  # test points per chunk (= per slot)
NSLOT = MC // CT          # 8 slots per core
NCHUNK = M // CT          # 64 chunks total
TT = 128                  # train points per tile
NTILE = N // TT           # 128 train tiles
C = 14                    # used contraction rows of the exponent matmul
CP = 128                  # padded contraction (sub-128 contraction throttles
                          # the PE clock to 1.2 GHz -- measured, not folklore)
NBAND = 16                # serpentine sort bands
EXP_CUT = 8.0             # drop blocks where all pairs have K < e^-EXP_CUT
GRP = 6                   # jobs per ACT instruction (3 PSUM banks)

_cache = {}


def _split2(v):
    hi = v.astype(ml_dtypes.bfloat16)
    lo = (v - hi.astype(np.float64)).astype(ml_dtypes.bfloat16)
    return hi, lo


def _split3(v):
    hi = v.astype(ml_dtypes.bfloat16)
    r = v - hi.astype(np.float64)
    mid = r.astype(ml_dtypes.bfloat16)
    lo = (r - mid.astype(np.float64)).astype(ml_dtypes.bfloat16)
    return hi, mid, lo


def _band_order(X, nbands):
    """Spatial sort: nbands equal-count x-bands, serpentine by y inside."""
    n = len(X)
    ox = np.argsort(X[:, 0], kind="stable")
    per = n // nbands
    order = np.empty(n, dtype=np.int64)
    for b in range(nbands):
        seg = ox[b * per : (b + 1) * per] if b < nbands - 1 else ox[b * per :]
        oy = seg[np.argsort(X[seg, 1], kind="stable")]
        if b % 2:
            oy = oy[::-1]
        order[b * per : b * per + len(oy)] = oy
    return order


def _point_bbox_d2(P, bmin, bmax):
    dx = np.maximum(0.0, np.maximum(bmin[None, :, 0] - P[:, None, 0],
                                    P[:, None, 0] - bmax[None, :, 0]))
    dy = np.maximum(0.0, np.maximum(bmin[None, :, 1] - P[:, None, 1],
                                    P[:, None, 1] - bmax[None, :, 1]))
    return dx * dx + dy * dy


def _make_AB(xs, xt, ell2):
    """Exponent factorization: exponent = A[:, i] . B[:, j] (train i, test j).

    Only the C=14 real contraction rows are materialized; the device zero-
    fills rows C..CP with a DVE memset (shipping the zero padding would be
    7x the DMA bytes).
    """
    n, m = len(xt), len(xs)
    x0h, x0l = _split2(xt[:, 0])
    x1h, x1l = _split2(xt[:, 1])
    pj = -(xt[:, 0] ** 2 + xt[:, 1] ** 2) / (2.0 * ell2)
    pjh, pjm, pjl = _split3(pj)
    ones = np.ones(n, dtype=ml_dtypes.bfloat16)
    A = np.stack(
        [ones, ones, ones, x0h, x0h, x0l, x0l, x1h, x1h, x1l, x1l, pjh, pjm, pjl]
    ).astype(ml_dtypes.bfloat16)

    T0 = -(xs[:, 0] ** 2 + xs[:, 1] ** 2) / (2.0 * ell2)
    T0h, T0m, T0l = _split3(T0)
    u0 = xs[:, 0] / ell2
    u0h, u0l = _split2(u0)
    u1 = xs[:, 1] / ell2
    u1h, u1l = _split2(u1)
    onesM = np.ones(m, dtype=ml_dtypes.bfloat16)
    B = np.stack(
        [T0h, T0m, T0l, u0h, u0l, u0h, u0l, u1h, u1l, u1h, u1l, onesM, onesM, onesM]
    ).astype(ml_dtypes.bfloat16)
    return A, B


def _build_program(bias, njob, slot_of_job):
    import concourse.mybir as mybir
    import concourse.tile as tile
    from concourse import bacc

    fp32 = mybir.dt.float32
    bf16 = mybir.dt.bfloat16

    ngrp = njob // GRP
    first_in_slot = {s: slot_of_job.index(s) for s in set(slot_of_job)}
    last_in_slot = {s: njob - 1 - slot_of_job[::-1].index(s)
                    for s in set(slot_of_job)}

    nc = bacc.Bacc(None, target_bir_lowering=False)
    A_d = nc.declare_dram_parameter("A", [C, njob * TT], bf16, isOutput=False)
    B_d = nc.declare_dram_parameter("B", [C, MC], bf16, isOutput=False)
    AL_d = nc.declare_dram_parameter("AL", [TT, njob * 4], bf16, isOutput=False)
    OUT_d = nc.declare_dram_parameter("out", [36, 1024], fp32, isOutput=True)

    with tile.TileContext(nc) as tc:
        with (
            tc.tile_pool(name="singles", bufs=1) as singles,
            tc.tile_pool(name="apool", bufs=1) as apool,
            tc.tile_pool(name="kpool", bufs=3) as kpool,
            tc.tile_pool(name="opool", bufs=1) as opool,
            tc.tile_pool(name="pse", bufs=2, space="PSUM") as pse,
            tc.tile_pool(name="psacc", bufs=1, space="PSUM") as psacc,
        ):
            sb_B = singles.tile([CP, MC], bf16)
            sb_AL = singles.tile([TT, njob * 4], bf16)
            # Rotating pre-zeroed staging for the A tiles: rows C:CP are
            # zeroed once per slot; each group's DMA then writes only the C
            # real rows (shipping the zero padding would be 9x the bytes, and
            # per-piece memsets at DVE 1x rate could not keep up).
            NSLOTS_A = 4
            a_slots = [apool.tile([CP, GRP * TT], bf16, name=f"a{i}")
                       for i in range(NSLOTS_A)]
            # B also ships thin (rows 0:C, 56KB vs 512KB): zero the full
            # region first, slot-0's columns separately so the first matmul
            # is not gated on the rest
            nc.vector.memset(sb_B[:, :CT], 0)
            nc.vector.memset(a_slots[0], 0)
            nc.gpsimd.memset(a_slots[1], 0)
            nc.sync.dma_start(out=sb_B[:C, :CT], in_=B_d[:, :CT])
            nc.gpsimd.dma_start(out=a_slots[0][:C, :],
                                in_=A_d[:, : GRP * TT])
            nc.scalar.dma_start(out=sb_AL[:, : 48 * 4], in_=AL_d[:, : 48 * 4])
            nc.vector.memset(a_slots[2], 0)
            nc.gpsimd.memset(a_slots[3], 0)
            nc.vector.memset(sb_B[:, CT:], 0)
            nc.sync.dma_start(out=sb_B[:C, CT:], in_=B_d[:, CT:])
            if njob > 48:
                nc.scalar.dma_start(out=sb_AL[:, 48 * 4 :], in_=AL_d[:, 48 * 4 :])

            # 8 slot accumulators packed in one 2-bank PSUM tile:
            # slot s -> partitions 32*(s%2)..+4, cols 256*(s//2)..+256
            acc = psacc.tile([36, 1024], fp32, name="acc")

            for g in range(ngrp):
                a6 = a_slots[g % NSLOTS_A]
                if g > 0:
                    eng = nc.gpsimd if g % 2 == 0 else nc.sync
                    eng.dma_start(
                        out=a6[:C, :],
                        in_=A_d[:, g * GRP * TT : (g + 1) * GRP * TT],
                    )
                e6 = pse.tile([128, GRP * CT], fp32)
                for q in range(GRP):
                    j = g * GRP + q
                    s = slot_of_job[j]
                    nc.tensor.matmul(
                        e6[:, q * CT : (q + 1) * CT],
                        lhsT=a6[:, q * TT : (q + 1) * TT],
                        rhs=sb_B[:, s * CT : (s + 1) * CT],
                        start=True,
                        stop=True,
                    )
                k6 = kpool.tile([128, GRP * CT], bf16)
                nc.scalar.activation(
                    k6, e6, mybir.ActivationFunctionType.Exp, bias=float(bias)
                )
                for q in range(GRP):
                    j = g * GRP + q
                    s = slot_of_job[j]
                    nc.tensor.matmul(
                        acc[32 * (s % 2) : 32 * (s % 2) + 4,
                            256 * (s // 2) : 256 * (s // 2) + 256],
                        lhsT=sb_AL[:, j * 4 : (j + 1) * 4],
                        rhs=k6[:, q * CT : (q + 1) * CT],
                        start=(j == first_in_slot[s]),
                        stop=(j == last_in_slot[s]),
                    )

            # two-half evacuation: the first half's deps (slots 0-3) clear
            # mid-kernel, hiding its copy + DMA under remaining compute
            for h in range(2):
                o = opool.tile([36, 512], fp32, name=f"o{h}")
                nc.vector.tensor_copy(o, acc[:, h * 512 : (h + 1) * 512])
                nc.sync.dma_start(
                    out=OUT_d[:, h * 512 : (h + 1) * 512], in_=o
                )
    nc.compile()
    return nc


def prepare(X_test, X_train, alpha, log_lengthscale, log_outputscale):
    """Host prep: sort, schedule, gather.  Returns (nc, in_maps, assemble, meta)."""
    ell = np.exp(np.float32(log_lengthscale))
    ell2 = np.float64(np.float32(ell) ** 2)
    sf = np.exp(np.float32(log_outputscale))
    sf2 = np.float64(np.float32(sf) ** 2)
    bias = np.float32(np.log(sf2))

    xs_all = X_test.astype(np.float64)
    xt_all = X_train.astype(np.float64)
    al_all = alpha.astype(np.float64)

    dcut = float(np.sqrt(2.0 * ell2 * EXP_CUT))

    ot = _band_order(xs_all, NBAND)
    orr = _band_order(xt_all, NBAND)
    xs = xs_all[ot]
    xt = xt_all[orr]
    al = al_all[orr]

    # --- block keep matrix: bbox prefilter + exact min pair distance --------
    tch = xs.reshape(NCHUNK, CT, 2)
    ttl = xt.reshape(NTILE, TT, 2)
    tmin, tmax = tch.min(1), tch.max(1)
    rmin, rmax = ttl.min(1), ttl.max(1)
    d2_tr = _point_bbox_d2(xt, tmin, tmax)
    d2_tr = d2_tr.reshape(NTILE, TT, NCHUNK).min(1)
    d2_te = _point_bbox_d2(xs, rmin, rmax)
    d2_te = d2_te.reshape(NCHUNK, CT, NTILE).min(1)
    pre = (d2_tr.T < dcut * dcut) & (d2_te < dcut * dcut)
    keep = np.zeros_like(pre)
    for ch in range(NCHUNK):
        idx = np.nonzero(pre[ch])[0]
        if len(idx) == 0:
            continue
        d2 = ((tch[ch][:, None, None, :] - ttl[idx][None, :, :, :]) ** 2).sum(-1)
        keep[ch, idx] = d2.min(axis=(0, 2)) < dcut * dcut
    cnt = keep.sum(1)

    # --- deal chunks to (core, slot); pad counts ---------------------------
    order = np.argsort(-cnt, kind="stable")
    T = [max(int(cnt[order[j * NCORES]]), 1) for j in range(NSLOT)]
    njob = sum(T)
    pad = (-njob) % GRP
    T[-1] += pad
    njob += pad

    chunk_of = np.empty((NCORES, NSLOT), dtype=np.int64)
    for j in range(NSLOT):
        for c in range(NCORES):
            chunk_of[c, j] = order[j * NCORES + c]

    # --- exponent factor matrices & alpha tiles (sorted order) -------------
    A, B = _make_AB(xs, xt, ell2)
    arh, arl = _split2(al[:, 0])
    aih, ail = _split2(al[:, 1])
    AL = np.stack([arh, arl, aih, ail], axis=1)      # (N, 4) bf16
    AL = np.ascontiguousarray(AL).reshape(NTILE, TT, 4)

    # --- per-core gathers --------------------------------------------------
    in_maps = []
    slot_of_job = []
    for j in range(NSLOT):
        slot_of_job += [j] * T[j]
    for c in range(NCORES):
        A_g = np.zeros((C, njob * TT), dtype=ml_dtypes.bfloat16)
        AL_g = np.zeros((TT, njob * 4), dtype=ml_dtypes.bfloat16)
        B_c = np.zeros((C, MC), dtype=ml_dtypes.bfloat16)
        ji = 0
        for j in range(NSLOT):
            ch = chunk_of[c, j]
            B_c[:, j * CT : (j + 1) * CT] = B[:, ch * CT : (ch + 1) * CT]
            tiles = np.nonzero(keep[ch])[0]
            for t in tiles:
                A_g[:, ji * TT : (ji + 1) * TT] = A[:, t * TT : (t + 1) * TT]
                AL_g[:, ji * 4 : (ji + 1) * 4] = AL[t]
                ji += 1
            ji += T[j] - len(tiles)  # dummy jobs stay zero
        assert ji == njob
        in_maps.append({"A": A_g, "B": B_c, "AL": AL_g})

    key = ("v9", float(bias), njob, tuple(T))
    if key not in _cache:
        _cache[key] = _build_program(bias, njob, slot_of_job)
    nc = _cache[key]

    def assemble(results):
        out = np.empty((M, 2), dtype=np.float32)
        for c in range(NCORES):
            o = results[c]["out"]  # (36, 1024)
            for j in range(NSLOT):
                g, q = j % 2, j // 2
                blk = o[32 * g : 32 * g + 4, 256 * q : 256 * q + 256]
                ch = chunk_of[c, j]
                rows = ot[ch * CT : (ch + 1) * CT]
                out[rows, 0] = blk[0] + blk[1]
                out[rows, 1] = blk[2] + blk[3]
        return out

    return nc, in_maps, assemble, slot_of_job


def simulate(nc_unused, in_maps, slot_of_job, bias):
    """Numpy emulation of the device program (for schedule/gather checks)."""
    results = []
    njob = len(slot_of_job)
    for c in range(NCORES):
        A_g = in_maps[c]["A"].astype(np.float32)
        B_c = in_maps[c]["B"].astype(np.float32)
        AL_g = in_maps[c]["AL"].astype(np.float32)
        o = np.zeros((36, 1024), dtype=np.float32)
        for j in range(njob):
            s = slot_of_job[j]
            e = (A_g[:, j * TT : (j + 1) * TT].T
                 @ B_c[: A_g.shape[0], s * CT : (s + 1) * CT])
            k = np.exp(e + bias).astype(ml_dtypes.bfloat16).astype(np.float32)
            contrib = AL_g[:, j * 4 : (j + 1) * 4].T @ k  # (4, CT)
            o[32 * (s % 2) : 32 * (s % 2) + 4,
              256 * (s // 2) : 256 * (s // 2) + 256] += contrib
        results.append({"out": o})
    return results


def kernel(X_test, X_train, alpha, log_lengthscale, log_outputscale):
    from concourse.bass_utils import run_bass_kernel_spmd

    nc, in_maps, assemble, _ = prepare(
        X_test, X_train, alpha, log_lengthscale, log_outputscale
    )
    res = run_bass_kernel_spmd(nc, in_maps, list(range(NCORES)))
    return assemble(res.results)
